# revision 1
# baseline (speedup 1.0000x reference)
"""Biaffine labeler kernel for 8 Trainium2 NeuronCores.

Computation (full shapes):
    dep  [2, 2048, 1024], head [2, 2049, 1024], head_indices [2, 2048]
    dep_label  = dep @ dep_W.T + dep_b                    [2, 2048, 512]
    selected   = (head gathered at head_indices) @ head_W.T + head_b
    logits[b,t,n] = dep_label[b,t,:] @ W[n] @ selected[b,t,:] + bias[n]

Sharding: data-parallel over (b, t): core c handles b = c // 4 and the
512-token range starting at (c % 4) * 512.  W / projections replicated.
The head shard each core receives is the 512 rows its tokens select
(the gather is resolved on the host as part of sharding), so no
on-device gather — and therefore no gpsimd SWDGE library — is needed;
its ~12us 16-queue ucode download was the old startup bottleneck.

Per-core device program (matmuls in bf16, fp32 PSUM accumulation):
    1. dep / selected-head shards and projection weights arrive
       host-pre-transposed/bf16 in device tile layout on the two HWDGE
       queues (sync + scalar) ahead of any W traffic
    2. projections run on PE; dep bias folds into the ACT psum->sbuf
       cast (per-partition bias AP), head bias via a K=1 matmul
    3. per label pair: W arrives host-pre-cast bf16 in device tile
       layout (1MB DMAs, 8KB per-partition rows, alternating HWDGE
       queues); A_n = dep_label @ W[n] on PE (4 K-chunks x 4 token
       chunks, N=512), one fused DVE scalar_tensor_tensor per chunk
       does logits[:, n] = sum_e A_n * selected (multiply + free-dim
       accumulate)
    4. logits += bias (broadcast via ones x biasn matmul), DMA out
"""

import sys

for _p in ("/opt/trn_rl_repo", "/root/.axon_site/_ro/trn_rl_repo"):
    if _p not in sys.path:
        sys.path.append(_p)

from contextlib import ExitStack

import ml_dtypes
import numpy as np

BF16NP = ml_dtypes.bfloat16

import concourse.bass as bass  # noqa: F401
import concourse.mybir as mybir
import concourse.tile as tile
from concourse import bacc
from concourse.bass_utils import run_bass_kernel_spmd

B, T, D = 2, 2048, 1024
E = 512            # label-space dim (D // 2)
NLAB = 50
NCORES = 8
TLOC = (B * T) // NCORES   # 512 tokens per core
TP = TLOC // 128           # 4 token chunks
DP = D // 128              # 8 contraction chunks for the projections
EP = E // 128              # 4 chunks of the label dim

F32 = mybir.dt.float32
BF16 = mybir.dt.bfloat16


def build_program():
    nc = bacc.Bacc("TRN2", target_bir_lowering=False, debug=False,
                   num_devices=NCORES)

    # operands split into d-chunk halves so each half can ride its own
    # HWDGE queue and compute can start when the first halves land
    HDP = DP // 2
    dep_A = nc.dram_tensor("dep_A", [128, HDP, TLOC], BF16,
                           kind="ExternalInput").ap()
    dep_B = nc.dram_tensor("dep_B", [128, HDP, TLOC], BF16,
                           kind="ExternalInput").ap()
    sel_A = nc.dram_tensor("sel_A", [128, HDP, TLOC], BF16,
                           kind="ExternalInput").ap()
    sel_B = nc.dram_tensor("sel_B", [128, HDP, TLOC], BF16,
                           kind="ExternalInput").ap()
    depW_A = nc.dram_tensor("depW_A", [128, HDP, E], BF16,
                            kind="ExternalInput").ap()
    depW_B = nc.dram_tensor("depW_B", [128, HDP, E], BF16,
                            kind="ExternalInput").ap()
    headW_A = nc.dram_tensor("headW_A", [128, HDP, E], BF16,
                             kind="ExternalInput").ap()
    headW_B = nc.dram_tensor("headW_B", [128, HDP, E], BF16,
                             kind="ExternalInput").ap()
    depb_c = nc.dram_tensor("depb_c", [128, EP], F32,
                            kind="ExternalInput").ap()
    headb = nc.dram_tensor("headb", [1, E], F32, kind="ExternalInput").ap()
    # host-pre-cast bf16 W in device tile layout: [n, p, d-chunk, e]
    Wb = nc.dram_tensor("Wb", [NLAB, 128, EP, E], BF16,
                        kind="ExternalInput").ap()
    biasn = nc.dram_tensor("biasn", [1, NLAB], F32, kind="ExternalInput").ap()
    logits = nc.dram_tensor("logits", [TLOC, NLAB], F32,
                            kind="ExternalOutput").ap()

    with tile.TileContext(nc) as tc, ExitStack() as ctx:
        # ---- persistent tiles (one pool, one slot per distinct tag) ----
        pp = ctx.enter_context(tc.tile_pool(name="persist", bufs=1))

        def ptile(shape, dtype, name):
            return pp.tile(shape, dtype, tag=name, name=name)

        ones_r = ptile([1, TLOC], BF16, "ones_r")
        stage_b = ptile([1, E], F32, "stage_b")
        depb_sb = ptile([128, EP], F32, "depb_sb")
        headb_sb = ptile([1, E], BF16, "headb_sb")
        biasn_f32 = ptile([1, NLAB], F32, "biasn_f32")
        biasn_sb = ptile([1, NLAB], BF16, "biasn_sb")
        bias_bc = ptile([128, NLAB], F32, "bias_bc")
        logit_out = ptile([128, TP, NLAB], F32, "logit_out")
        dep_lT = ptile([128, EP, TLOC], BF16, "dep_lT")   # [e, tok]
        sel_sb = ptile([128, TP, E], BF16, "sel_sb")      # [tok, e]
        # one tile PER DMA half so phase-A matmuls depend only on the
        # A-half transfers (tile-granular dependency tracking)
        HDP2 = DP // 2
        dep_sA = ptile([128, HDP2, TLOC], BF16, "dep_sA")
        dep_sB = ptile([128, HDP2, TLOC], BF16, "dep_sB")
        sel_rA = ptile([128, HDP2, TLOC], BF16, "sel_rA")
        sel_rB = ptile([128, HDP2, TLOC], BF16, "sel_rB")
        depW_sA = ptile([128, HDP2, E], BF16, "depW_sA")
        depW_sB = ptile([128, HDP2, E], BF16, "depW_sB")
        headW_sA = ptile([128, HDP2, E], BF16, "headW_sA")
        headW_sB = ptile([128, HDP2, E], BF16, "headW_sB")
        logit_sb = ptile([128, TP, NLAB], F32, "logit_sb")

        w_pool = ctx.enter_context(tc.tile_pool(name="wn", bufs=4))
        dead_pool = ctx.enter_context(tc.tile_pool(name="dead", bufs=2))

        # startup-critical loads, interleaved across the two HWDGE queues
        # in the order the PE will need them: dep halves first (dep
        # projection), then selected-head / head weights, then W labels
        nc.sync.dma_start(dep_sA[:], dep_A)
        nc.scalar.dma_start(depW_sA[:], depW_A)
        nc.sync.dma_start(depW_sB[:], depW_B)
        nc.scalar.dma_start(dep_sB[:], dep_B)
        nc.scalar.dma_start(depb_sb[:], depb_c)
        nc.vector.memset(ones_r[:], 1.0)

        ps_pool = ctx.enter_context(
            tc.tile_pool(name="ps", bufs=6, space="PSUM"))

        # PE warmup while the dep DMAs land: dataless K=1 matmuls ramp
        # the PE out of its low-power pstate so the projections run at
        # full clock from their first instruction
        for _ in range(10):
            psw = ps_pool.tile([128, 512], F32, tag="ps")
            nc.tensor.matmul(psw[:], ones_r[:1, :128], ones_r[:1, :],
                             start=True, stop=True)

        # dep projection -> dep_labelT [e, tok], contraction phased over
        # the two data halves so phase A starts when half the bytes have
        # landed; dep bias folds into the ACT psum->sbuf cast
        dpsp = []
        for i in range(EP):
            psp = ps_pool.tile([128, 512], F32, tag="ps")
            dpsp.append(psp)
            for j in range(HDP2):
                nc.tensor.matmul(psp[:],
                                 depW_sA[:, j, i * 128:(i + 1) * 128],
                                 dep_sA[:, j, :],
                                 start=(j == 0), stop=False)
        for i in range(EP):
            psp = dpsp[i]
            for j in range(HDP2):
                nc.tensor.matmul(psp[:],
                                 depW_sB[:, j, i * 128:(i + 1) * 128],
                                 dep_sB[:, j, :],
                                 start=False, stop=(j == HDP2 - 1))
            nc.scalar.activation(dep_lT[:, i, :], psp[:],
                                 mybir.ActivationFunctionType.Identity,
                                 bias=depb_sb[:, i:i + 1])

        # loads for the head projection, the bias tail, and the W stream
        nc.scalar.dma_start(sel_rA[:], sel_A)
        nc.sync.dma_start(headW_sA[:], headW_A)
        nc.sync.dma_start(sel_rB[:], sel_B)
        nc.scalar.dma_start(headW_sB[:], headW_B)
        nc.scalar.dma_start(stage_b[:], headb)
        nc.scalar.copy(headb_sb[:], stage_b[:])
        nc.scalar.dma_start(biasn_f32[:], biasn)
        nc.scalar.copy(biasn_sb[:], biasn_f32[:])

        # head projection of pre-gathered rows -> selected [tok, e],
        # phased the same way; head bias via a K=1 matmul at group end
        hpsp = []
        for i in range(TP):
            psp = ps_pool.tile([128, 512], F32, tag="ps")
            hpsp.append(psp)
            for j in range(HDP2):
                nc.tensor.matmul(psp[:],
                                 sel_rA[:, j, i * 128:(i + 1) * 128],
                                 headW_sA[:, j, :],
                                 start=(j == 0), stop=False)
        for i in range(TP):
            psp = hpsp[i]
            for j in range(HDP2):
                nc.tensor.matmul(psp[:],
                                 sel_rB[:, j, i * 128:(i + 1) * 128],
                                 headW_sB[:, j, :],
                                 start=False, stop=False)
            nc.tensor.matmul(psp[:], ones_r[:, :128], headb_sb[:],
                             start=False, stop=True)
            nc.scalar.copy(sel_sb[:, i, :], psp[:])

        # bias[n] broadcast across partitions (needed only at the end):
        # ones[128] x biasn
        psb = ps_pool.tile([128, 512], F32, tag="ps")
        nc.tensor.matmul(psb[:, :NLAB], ones_r[:, :128], biasn_sb[:],
                         start=True, stop=True)
        nc.scalar.copy(bias_bc[:], psb[:, :NLAB])

        # biaffine main loop: per-token-chunk PSUM tiles (fine pipelining)
        for n in range(NLAB):
            wt = w_pool.tile([128, EP, E], BF16, tag="wn")
            eng = nc.sync if n % 2 == 0 else nc.scalar
            eng.dma_start(wt[:], Wb[n])
            for i in range(TP):
                psa = ps_pool.tile([128, 512], F32, tag="ps")
                for j in range(EP):
                    nc.tensor.matmul(psa[:],
                                     dep_lT[:, j, i * 128:(i + 1) * 128],
                                     wt[:, j, :],
                                     start=(j == 0), stop=(j == EP - 1))
                dead = dead_pool.tile([128, E], BF16, tag="dead")
                nc.vector.scalar_tensor_tensor(
                    out=dead[:], in0=psa[:], scalar=1.0,
                    in1=sel_sb[:, i, :],
                    op0=mybir.AluOpType.mult, op1=mybir.AluOpType.mult,
                    accum_out=logit_sb[:, i, n:n + 1])

        # per-chunk bias add + store, so each chunk ships as soon as its
        # last label finishes instead of waiting for the whole tensor
        logits_r = logits.rearrange("(i p) n -> p i n", p=128)
        for i in range(TP):
            nc.vector.tensor_add(logit_out[:, i, :], logit_sb[:, i, :],
                                 bias_bc[:])
            nc.sync.dma_start(logits_r[:, i, :], logit_out[:, i, :])

    nc.compile()
    return nc


_NC_CACHE = []


def _get_program():
    if not _NC_CACHE:
        _NC_CACHE.append(build_program())
    return _NC_CACHE[0]


def make_in_maps(dep, head, head_indices, dep_W, dep_b, head_W, head_b, W,
                 bias):
    dep = np.asarray(dep, dtype=np.float32)
    head = np.asarray(head, dtype=np.float32)
    idx = np.asarray(head_indices)
    def dev_layout(a):
        # [x, 1024] operand -> transposed bf16 tile layout [128, 8, x]
        at = np.asarray(a, dtype=np.float32).T.astype(BF16NP)
        return np.ascontiguousarray(
            at.reshape(DP, 128, at.shape[1]).transpose(1, 0, 2))

    # W -> bf16 device tile layout [n, p, j, e] with d = j*128 + p
    Wb = np.ascontiguousarray(
        np.asarray(W, dtype=np.float32).astype(BF16NP)
        .reshape(NLAB, EP, 128, E).transpose(0, 2, 1, 3))

    def halves(a):
        h = DP // 2
        return (np.ascontiguousarray(a[:, :h]),
                np.ascontiguousarray(a[:, h:]))

    depW_A, depW_B = halves(dev_layout(dep_W))
    headW_A, headW_B = halves(dev_layout(head_W))
    shared = {
        "depW_A": depW_A, "depW_B": depW_B,
        "headW_A": headW_A, "headW_B": headW_B,
        # dep bias as per-partition columns: depb_c[p, i] = dep_b[i*128+p]
        "depb_c": np.ascontiguousarray(
            np.asarray(dep_b, dtype=np.float32).reshape(EP, 128).T),
        "headb": np.ascontiguousarray(head_b, dtype=np.float32).reshape(1, E),
        "Wb": Wb,
        "biasn": np.ascontiguousarray(bias, dtype=np.float32).reshape(1, NLAB),
    }
    in_maps = []
    cores_per_b = NCORES // B
    for c in range(NCORES):
        b = c // cores_per_b
        t0 = (c % cores_per_b) * TLOC
        dep_A, dep_B = halves(dev_layout(dep[b, t0:t0 + TLOC]))
        # head shard for this core = the rows its tokens select
        sel_A, sel_B = halves(dev_layout(head[b][idx[b, t0:t0 + TLOC]]))
        in_maps.append({
            "dep_A": dep_A, "dep_B": dep_B,
            "sel_A": sel_A, "sel_B": sel_B,
            **shared,
        })
    return in_maps


def run_sharded(inputs, trace=False):
    """Run the SPMD kernel; returns (full_logits, BassKernelResults)."""
    nc = _get_program()
    in_maps = make_in_maps(
        inputs["dep"], inputs["head"], inputs["head_indices"],
        inputs["dep_W"], inputs["dep_b"], inputs["head_W"],
        inputs["head_b"], inputs["W"], inputs["bias"])
    last_err = None
    for attempt in range(3):
        try:
            res = run_bass_kernel_spmd(nc, in_maps, list(range(NCORES)),
                                       trace=trace)
            break
        except Exception as e:  # transient NRT_EXEC device errors
            last_err = e
            if attempt == 2:
                raise
            import time
            time.sleep(5)
    out = np.empty((B, T, NLAB), dtype=np.float32)
    cores_per_b = NCORES // B
    for c in range(NCORES):
        b = c // cores_per_b
        t0 = (c % cores_per_b) * TLOC
        out[b, t0:t0 + TLOC] = res.results[c]["logits"]
    return out, res


def kernel(dep, head, head_indices, mask, dep_W, dep_b, head_W, head_b, W,
           bias):
    out, _ = run_sharded({
        "dep": dep, "head": head, "head_indices": head_indices,
        "dep_W": dep_W, "dep_b": dep_b, "head_W": head_W,
        "head_b": head_b, "W": W, "bias": bias,
    })
    return out

